# revision 16
# baseline (speedup 1.0000x reference)
"""ChildSumTreeLSTM on a complete binary tree (N=8191), 8-core Trainium2.

v3: heap-ordered tree = 7 top nodes + 8 independent 1023-node subtrees,
one per NeuronCore. Per core, feature-major [256 feats x cols] layout with
col = subtree-local heap index (level l at cols [2^l, 2^(l+1)), leaves at
[512,1024)).

- x-projections in fp8e4m3 DoubleRow matmuls (K=256 in one instruction),
  computed just-in-time into the same PSUM accumulation group as each
  level's bf16 h-matmuls (scan weights pre-scaled x4096 to match the fp8
  input scaling; activations descale by 1/4096 and add biases via ports).
- f-gate x-terms use a host-duplicated x tensor (x8d[c] = x8[c//2]).
- Input DMA split so the leaf half arrives first; leaf matmuls start
  immediately.
- Dummy matmuls keep the PE busy through activation windows so it stays
  at the fast p-state.
- Device computes leaves + levels 256/128; the top of each subtree
  (<=64, 1023 nodes total + global top 7) finishes on host, vectorized.
"""

import numpy as np

import concourse.bass as bass
import concourse.tile as tile
from concourse import mybir
from concourse.bass_utils import run_bass_kernel_spmd

F32 = mybir.dt.float32
BF16 = mybir.dt.bfloat16
FP8 = mybir.dt.float8e4
AFT = mybir.ActivationFunctionType
DR = mybir.MatmulPerfMode.DoubleRow

N_NODES = 8191
D = 256
M = 256
NCOL = 1024
SUB_LEVELS = 10
DESCALE = 1.0 / 4096.0  # x8 = 128*x, wc8 = 32*Wc, wsc = 4096*W
DEV_LEVELS = (256, 128)  # internal levels computed on device
BOUND = 128              # boundary level emitted to host


def _split_excess_waits(nc, max_waits=1):
    """walrus in this container allows only 1 sync-wait per instruction.

    Tile can attach several; hoist the extras onto injected same-engine NOPs
    immediately preceding the instruction (same blocking semantics)."""
    k = 0
    for f in nc.m.functions:
        for bb in f.blocks:
            out = []
            changed = False
            for ins in bb.instructions:
                si = ins.sync_info
                w = list(si.on_wait) if si and si.on_wait else []
                if len(w) > max_waits:
                    hoist, keep = w[:-max_waits], w[-max_waits:]
                    for sw in hoist:
                        nop = mybir.InstNoOp(name=f"whoist{k}", ins=[], outs=[])
                        k += 1
                        nop.engine = ins.engine
                        nop.sync_info = mybir.SyncInfo(on_wait=[sw], on_update=[])
                        out.append(nop)
                    si.on_wait = keep
                    changed = True
                out.append(ins)
            if changed:
                bb.instructions = out


def _build_module():
    nc = bass.Bass(num_devices=8)

    # head8: wc8 [0:1024] | x8 leaf half [1024:1536] (leaf cols 512..1023)
    head8 = nc.dram_tensor("head8", [128, 2, 1536], FP8, kind="ExternalInput")
    # rest8: x8 internal cols 0..511 [0:512] | x8d cols 128..1023 [512:1408]
    rest8 = nc.dram_tensor("rest8", [128, 2, 1408], FP8, kind="ExternalInput")
    wsc = nc.dram_tensor("wsc", [128, 2, NCOL], BF16, kind="ExternalInput")
    biasd = nc.dram_tensor("biasd", [128, 16], F32, kind="ExternalInput")
    out_c = nc.dram_tensor("out_c", [128, 2 * BOUND], BF16, kind="ExternalOutput")
    out_h = nc.dram_tensor("out_h", [128, 2 * BOUND], BF16, kind="ExternalOutput")

    with tile.TileContext(nc) as tc:
        with (
            tc.tile_pool(name="consts", bufs=1) as consts,
            tc.tile_pool(name="tmps", bufs=2) as tmps,
            tc.tile_pool(name="spsum", bufs=1, space="PSUM") as spsum,
        ):
            sb_h8 = consts.tile([128, 2, 1536], FP8, tag="h8")
            nc.sync.dma_start(out=sb_h8[:], in_=head8[:])
            sb_b = consts.tile([128, 16], F32, tag="bias")
            nc.sync.dma_start(out=sb_b[:], in_=biasd[:])
            sb_r8 = consts.tile([128, 2, 1408], FP8, tag="r8")
            nc.sync.dma_start(out=sb_r8[:], in_=rest8[:])
            sb_wsc = consts.tile([128, 2, NCOL], BF16, tag="wsc")
            nc.sync.dma_start(out=sb_wsc[:], in_=wsc[:])

            wc8 = sb_h8[:, :, 0:1024]
            x8leaf = sb_h8[:, :, 1024:1536]    # leaf cols 512..1023
            x8int = sb_r8[:, :, 0:512]         # cols 0..511

            def x8d_ap(lo, hi):  # duplicated-parent cols lo..hi (128<=lo)
                return sb_r8[:, :, 512 + lo - 128 : 512 + hi - 128]

            H = consts.tile([128, 2, NCOL], BF16, tag="H")
            C = consts.tile([128, 2, NCOL], BF16, tag="C")

            # preload the sigmoid/tanh ACT table during the input DMA
            # (values are garbage; only the table-load side effect matters)
            warm = consts.tile([128, 1], F32, tag="warm")
            nc.scalar.activation(warm[:], H[:, 0, 0:1], AFT.Sigmoid)
            nc.scalar.activation(warm[:], H[:, 0, 0:1], AFT.Tanh)

            # leaf gate tiles (cols 512..1023)
            l_si = consts.tile([128, 2, 512], BF16, tag="l_si")
            l_so = consts.tile([128, 2, 512], BF16, tag="l_so")
            l_tu = consts.tile([128, 2, 512], BF16, tag="l_tu")
            l_fc = consts.tile([128, 2, 512], BF16, tag="l_fc")
            l_iu = consts.tile([128, 2, 512], BF16, tag="l_iu")
            l_tc = consts.tile([128, 2, 512], BF16, tag="l_tc")
            hs = consts.tile([128, 2, 256], BF16, tag="hs_l")

            def ps_iou_tile(tag, name):
                return spsum.tile([128, 2, 256], F32, tag=tag, bufs=2, name=name)

            def ps_f_tile(name):
                return spsum.tile([128, 512], F32, tag="pf", bufs=2, name=name)

            def dummy_mms(count, target_ap):
                # PE p-state keepalive: throwaway fp8 MMs into a PSUM region
                # that the next real group resets with start=True.
                for _ in range(count):
                    nc.tensor.matmul(
                        target_ap, wc8[:, :, 0:128], wc8[:, :, 0:512],
                        start=True, stop=True, perf_mode=DR,
                        skip_group_check=True,
                    )

            # ---- leaf phase ----
            # F: 0,1=i  2,3=o  4,5=u  6,7=fx   (sub = F%2 feature half)
            leaf_ps = {}
            order = (0, 4, 6, 2, 1, 5, 7, 3)  # i0,u0,f0,o0, i1,u1,f1,o1
            for F in order:
                if F < 6:
                    ps = ps_iou_tile(["pi", "pi", "po", "po", "pu", "pu"][F], f"lps{F}")
                    ps_ap = ps[:, :, :]
                else:
                    ps = ps_f_tile(f"lps{F}")
                    ps_ap = ps[:, :]
                nc.tensor.matmul(
                    ps_ap, wc8[:, :, 128 * F : 128 * (F + 1)], x8leaf[:],
                    start=True, stop=True, perf_mode=DR,
                )
                leaf_ps[F] = ps_ap
            gate_of = {0: l_si, 1: l_si, 2: l_so, 3: l_so, 4: l_tu, 5: l_tu,
                       6: l_fc, 7: l_fc}
            func_of = {0: AFT.Sigmoid, 1: AFT.Sigmoid, 2: AFT.Sigmoid,
                       3: AFT.Sigmoid, 4: AFT.Tanh, 5: AFT.Tanh,
                       6: AFT.Sigmoid, 7: AFT.Sigmoid}
            bcol_of = {0: 6, 1: 7, 2: 8, 3: 9, 4: 10, 5: 11, 6: 14, 7: 15}

            def leaf_act(F):
                nc.scalar.activation(
                    gate_of[F][:, F % 2, :], leaf_ps[F], func_of[F],
                    bias=sb_b[:, bcol_of[F] : bcol_of[F] + 1], scale=DESCALE,
                )

            # j=0 chain
            for F in (0, 4, 6):
                leaf_act(F)
            nc.vector.tensor_mul(l_iu[:, 0], l_si[:, 0], l_tu[:, 0])
            nc.vector.tensor_add(C[:, 0, 512:1024], l_iu[:, 0], l_fc[:, 0])
            leaf_act(2)
            nc.scalar.activation(l_tc[:, 0, :], C[:, 0, 512:1024], AFT.Tanh)
            nc.vector.tensor_mul(H[:, 0, 512:1024], l_so[:, 0], l_tc[:, 0])
            nc.vector.tensor_add(
                hs[:, 0, :], H[:, 0, 512:1024:2], H[:, 0, 513:1024:2]
            )
            # j=1 chain (adds on gpsimd to keep DVE free)
            for F in (1, 5, 7):
                leaf_act(F)
            nc.vector.tensor_mul(l_iu[:, 1], l_si[:, 1], l_tu[:, 1])
            nc.vector.tensor_add(C[:, 1, 512:1024], l_iu[:, 1], l_fc[:, 1])
            leaf_act(3)
            nc.scalar.activation(l_tc[:, 1, :], C[:, 1, 512:1024], AFT.Tanh)
            nc.vector.tensor_mul(H[:, 1, 512:1024], l_so[:, 1], l_tc[:, 1])
            nc.vector.tensor_add(
                hs[:, 1, :], H[:, 1, 512:1024:2], H[:, 1, 513:1024:2]
            )

            # ---- internal levels ----
            first = True
            for n in DEV_LEVELS:
                a, b2 = n, 2 * n          # parent cols
                ca, cb = 2 * n, 4 * n     # child cols

                if not first:
                    nc.vector.tensor_add(
                        hs[:, 0, :n], H[:, 0, ca:cb:2], H[:, 0, ca + 1 : cb : 2]
                    )
                    nc.vector.tensor_add(
                        hs[:, 1, :n], H[:, 1, ca:cb:2], H[:, 1, ca + 1 : cb : 2]
                    )

                # JIT x-projections (no H dependency: run during prior acts)
                ps_f = []
                for h in range(2):
                    ps = ps_f_tile(f"psf{h}_{n}")
                    Fb = 768 + 128 * h
                    if h == 0:
                        dummy_mms(10, ps[:, 0:512])
                    nc.tensor.matmul(
                        ps[:, : 2 * n], wc8[:, :, Fb : Fb + 128], x8d_ap(ca, cb),
                        start=True, stop=False, perf_mode=DR,
                    )
                    ps_f.append(ps)
                ps_iou = []
                for pair in range(3):
                    ps = ps_iou_tile(["pi", "po", "pu"][pair], f"ps{'iou'[pair]}_{n}")
                    for sub in range(2):
                        F = 2 * pair + sub
                        nc.tensor.matmul(
                            ps[:, sub, :n],
                            wc8[:, :, 128 * F : 128 * (F + 1)],
                            x8int[:, :, a:b2],
                            start=True, stop=False, perf_mode=DR,
                        )
                    ps_iou.append(ps)

                # h-matmuls: all j=0 first, then j=1 (j=0 leaf chain is ready
                # earlier); within j: f, u, i, o
                for j in range(2):
                    last = j == 1
                    for h in range(2):
                        Fb = 768 + 128 * h
                        nc.tensor.matmul(
                            ps_f[h][:, : 2 * n],
                            sb_wsc[:, j, Fb : Fb + 128],
                            H[:, j, ca:cb],
                            start=False, stop=last,
                        )
                    for pair in (2, 0, 1):  # u, i, o
                        for sub in range(2):
                            F = 2 * pair + sub
                            nc.tensor.matmul(
                                ps_iou[pair][:, sub, :n],
                                sb_wsc[:, j, 128 * F : 128 * (F + 1)],
                                hs[:, j, :n],
                                start=False, stop=last,
                            )

                # acts + cell, sub-split so the c0/tanh_c0/h0 chain runs
                # under the sub-1 and o activations
                t_f = tmps.tile([128, 2, 512], BF16, tag="t_f")
                t_si = tmps.tile([128, 2, 256], BF16, tag="t_si")
                t_so = tmps.tile([128, 2, 256], BF16, tag="t_so")
                t_tu = tmps.tile([128, 2, 256], BF16, tag="t_tu")
                g = tmps.tile([128, 2, 512], BF16, tag="g")
                fc = tmps.tile([128, 2, 256], BF16, tag="fc")
                iu = tmps.tile([128, 2, 256], BF16, tag="iu")
                t_tc = tmps.tile([128, 2, 256], BF16, tag="t_tc")

                def act_f(h):
                    nc.scalar.activation(
                        t_f[:, h, : 2 * n], ps_f[h][:, : 2 * n], AFT.Sigmoid,
                        bias=sb_b[:, 12 + h : 13 + h], scale=DESCALE,
                    )

                def act_iou(pair, gate, func, sub):
                    F = 2 * pair + sub
                    nc.scalar.activation(
                        gate[:, sub, :n], ps_iou[pair][:, sub, :n], func,
                        bias=sb_b[:, F : F + 1], scale=DESCALE,
                    )

                def gfc(s):
                    nc.vector.tensor_mul(
                        g[:, s, : 2 * n], t_f[:, s, : 2 * n], C[:, s, ca:cb]
                    )
                    nc.vector.tensor_add(
                        fc[:, s, :n], g[:, s, 0 : 2 * n : 2], g[:, s, 1 : 2 * n : 2]
                    )

                def iuc(s):
                    nc.vector.tensor_mul(iu[:, s, :n], t_si[:, s, :n], t_tu[:, s, :n])
                    nc.vector.tensor_add(C[:, s, a:b2], iu[:, s, :n], fc[:, s, :n])

                act_f(0)
                act_f(1)
                gfc(0)
                act_iou(2, t_tu, AFT.Tanh, 0)
                act_iou(0, t_si, AFT.Sigmoid, 0)
                gfc(1)
                iuc(0)
                act_iou(2, t_tu, AFT.Tanh, 1)
                act_iou(0, t_si, AFT.Sigmoid, 1)
                iuc(1)
                act_iou(1, t_so, AFT.Sigmoid, 0)
                nc.scalar.activation(t_tc[:, 0, :n], C[:, 0, a:b2], AFT.Tanh)
                nc.vector.tensor_mul(H[:, 0, a:b2], t_so[:, 0, :n], t_tc[:, 0, :n])
                if n == BOUND:
                    nc.sync.dma_start(out=out_c[:, :], in_=C[:, :, BOUND : 2 * BOUND])
                act_iou(1, t_so, AFT.Sigmoid, 1)
                nc.scalar.activation(t_tc[:, 1, :n], C[:, 1, a:b2], AFT.Tanh)
                nc.vector.tensor_mul(H[:, 1, a:b2], t_so[:, 1, :n], t_tc[:, 1, :n])
                if n == BOUND:
                    nc.sync.dma_start(
                        out=out_h[:, 0:BOUND], in_=H[:, 0, BOUND : 2 * BOUND]
                    )
                first = False

            # ---- emit boundary ----
            nc.gpsimd.dma_start(
                out=out_h[:, BOUND : 2 * BOUND], in_=H[:, 1, BOUND : 2 * BOUND]
            )

    _split_excess_waits(nc)
    return nc


_NC_CACHE = None


def _get_module():
    global _NC_CACHE
    if _NC_CACHE is None:
        _NC_CACHE = _build_module()
    return _NC_CACHE


def _expected_children():
    j = (N_NODES - 1) - np.arange(N_NODES)
    internal = (2 * j + 1) < N_NODES
    ch0 = (N_NODES - 1) - (2 * j + 1)
    ch1 = (N_NODES - 1) - (2 * j + 2)
    children = np.stack(
        [np.where(internal, ch0, 0), np.where(internal, ch1, 0)], axis=1
    ).astype(np.int32)
    mask = np.stack([internal, internal], axis=1)
    return children, mask


def _reference_numpy(emb, W_ioux, b_ioux, W_iouh, b_iouh, W_fx, b_fx, W_fh, b_fh,
                     ops, children, child_mask):
    # generic fallback (matches reference.py) for unexpected tree structure
    def sigmoid(v):
        return 1.0 / (1.0 + np.exp(-v))

    N = ops.shape[0]
    Md = W_fh.shape[0]
    x = emb[ops]
    iou_x = x @ W_ioux.T + b_ioux
    fx_all = x @ W_fx.T + b_fx
    ones = np.ones((Md,), np.float32)
    leaf_fh = ones @ W_fh.T + b_fh
    maskf = child_mask.astype(np.float32)
    c_arr = np.zeros((N, Md), np.float32)
    h_arr = np.zeros((N, Md), np.float32)
    for t in range(N):
        idx = children[t]
        m = maskf[t][:, None]
        ch_c = c_arr[idx] * m
        ch_h = h_arr[idx] * m
        is_leaf = maskf[t].sum() == 0
        h_sum = ones if is_leaf else ch_h.sum(0)
        iou = iou_x[t] + h_sum @ W_iouh.T + b_iouh
        i, o, u = np.split(iou, 3)
        i, o, u = sigmoid(i), sigmoid(o), np.tanh(u)
        f = sigmoid(ch_h @ W_fh.T + b_fh + fx_all[t])
        fc_int = (f * ch_c).sum(0)
        fc_leaf = sigmoid(leaf_fh + fx_all[t])
        fc = fc_leaf if is_leaf else fc_int
        c = i * u + fc
        h = o * np.tanh(c)
        c_arr[t] = c
        h_arr[t] = h
    return np.stack([c_arr[N - 1], h_arr[N - 1]])


def _col_index_for_core(k):
    # col 0 pad; cols 1..1023: subtree-local heap order shifted by +1
    # (level l at cols [2^l, 2^(l+1)), leaves exactly at [512, 1024))
    idx = np.zeros(NCOL, np.int64)
    for l in range(SUB_LEVELS):
        n = 1 << l
        g0 = (1 << (3 + l)) - 1 + k * n
        idx[n : 2 * n] = g0 + np.arange(n)
    return idx


def _pack_fm(mat, dtype):
    # mat [cols, 256] -> [128, 2, cols]: out[p, j, c] = mat[c, j*128+p]
    cols = mat.shape[0]
    return np.ascontiguousarray(
        mat.T.reshape(2, 128, cols).transpose(1, 0, 2)
    ).astype(dtype)


def kernel(**inputs):
    emb = np.asarray(inputs["emb"], np.float32)
    W_ioux = np.asarray(inputs["W_ioux"], np.float32)
    b_ioux = np.asarray(inputs["b_ioux"], np.float32)
    W_iouh = np.asarray(inputs["W_iouh"], np.float32)
    b_iouh = np.asarray(inputs["b_iouh"], np.float32)
    W_fx = np.asarray(inputs["W_fx"], np.float32)
    b_fx = np.asarray(inputs["b_fx"], np.float32)
    W_fh = np.asarray(inputs["W_fh"], np.float32)
    b_fh = np.asarray(inputs["b_fh"], np.float32)
    ops = np.asarray(inputs["ops"], np.int32)
    children = np.asarray(inputs["children"], np.int32)
    child_mask = np.asarray(inputs["child_mask"])

    exp_children, exp_mask = _expected_children()
    if (
        ops.shape[0] != N_NODES
        or not np.array_equal(children, exp_children)
        or not np.array_equal(child_mask.astype(bool), exp_mask)
    ):
        return _reference_numpy(
            emb, W_ioux, b_ioux, W_iouh, b_iouh, W_fx, b_fx, W_fh, b_fh,
            ops, children, child_mask,
        )

    import ml_dtypes

    fp8 = ml_dtypes.float8_e4m3
    bf16 = ml_dtypes.bfloat16

    # ---- host prep ----
    x = emb[ops]          # [8191, 256] topo order
    x_heap = x[::-1]      # heap order (topo t = N-1-j)

    Wc = np.concatenate([W_ioux, W_fx], 0)       # [1024, 256]
    Ws = np.concatenate([W_iouh, W_fh], 0)       # [1024, 256]
    wc8 = _pack_fm(32.0 * Wc, fp8)               # [128, 2, 1024]
    wsc = _pack_fm(4096.0 * Ws, bf16)

    bias = np.zeros((128, 16), np.float32)
    bias[:, 0:6] = (b_ioux + b_iouh).reshape(6, 128).T
    bias[:, 6:12] = (b_ioux + W_iouh.sum(1) + b_iouh).reshape(6, 128).T
    bias[:, 12:14] = (b_fx + b_fh).reshape(2, 128).T
    bias[:, 14:16] = (b_fx + W_fh.sum(1) + b_fh).reshape(2, 128).T

    common = {"wsc": wsc, "biasd": bias}
    in_maps = []
    for k in range(8):
        idx = _col_index_for_core(k)
        xv = x_heap[idx]                          # [1024, 256]
        x8 = _pack_fm(128.0 * xv, fp8)
        head8 = np.ascontiguousarray(
            np.concatenate([wc8, x8[:, :, 512:1024]], axis=2)
        )
        x8d = x8[:, :, np.arange(128, 1024) // 2]
        rest8 = np.ascontiguousarray(
            np.concatenate([x8[:, :, 0:512], x8d], axis=2)
        )
        in_maps.append({"head8": head8, "rest8": rest8, **common})

    global _LAST_IN_MAPS
    _LAST_IN_MAPS = in_maps
    nc = _get_module()
    res = run_bass_kernel_spmd(nc, in_maps, list(range(8)))

    # ---- host: subtree levels 64..1 + global top 7 ----
    def sigmoid(v):
        return 1.0 / (1.0 + np.exp(-v))

    # unpack boundary: [128, 2*BOUND] -> [BOUND nodes, 256 feats]
    C_loc = np.zeros((8, 2 * BOUND, M), np.float32)
    H_loc = np.zeros((8, 2 * BOUND, M), np.float32)
    for k in range(8):
        rc = res.results[k]["out_c"]
        rh = res.results[k]["out_h"].astype(np.float32)
        C_loc[k, BOUND:] = rc.reshape(128, 2, BOUND).transpose(2, 1, 0).reshape(BOUND, M)
        H_loc[k, BOUND:] = rh.reshape(128, 2, BOUND).transpose(2, 1, 0).reshape(BOUND, M)

    # x-projections for host nodes (cols 1..BOUND-1 per core + global top 7)
    nb = BOUND - 1
    idx_all = np.stack([_col_index_for_core(k)[1:BOUND] for k in range(8)])
    x_host = x_heap[idx_all.reshape(-1)].astype(np.float32)
    iou_xh = (x_host @ W_ioux.T + b_ioux + b_iouh).reshape(8, nb, 3 * M)
    fx_h = (x_host @ W_fx.T + b_fx + b_fh).reshape(8, nb, M)

    n = BOUND // 2
    while n >= 1:
        ch_h = H_loc[:, 2 * n : 4 * n]            # [8, 2n, 256]
        ch_c = C_loc[:, 2 * n : 4 * n]
        hsum = ch_h[:, 0::2] + ch_h[:, 1::2]      # [8, n, 256]
        iou = iou_xh[:, n - 1 : 2 * n - 1] + hsum @ W_iouh.T
        i_g = sigmoid(iou[:, :, :M])
        o_g = sigmoid(iou[:, :, M : 2 * M])
        u_g = np.tanh(iou[:, :, 2 * M :])
        fxd = np.repeat(fx_h[:, n - 1 : 2 * n - 1], 2, axis=1)
        f = sigmoid(ch_h @ W_fh.T + fxd)
        gfc = f * ch_c
        fcs = gfc[:, 0::2] + gfc[:, 1::2]
        c = i_g * u_g + fcs
        C_loc[:, n : 2 * n] = c
        H_loc[:, n : 2 * n] = o_g * np.tanh(c)
        n //= 2

    # global top 15: nodes 7..14 are the subtree roots (core k -> 7+k)
    x_top = x_heap[0:7].astype(np.float32)
    iou_x7 = x_top @ W_ioux.T + b_ioux + b_iouh
    fx7 = x_top @ W_fx.T + b_fx + b_fh
    c_arr = np.zeros((15, M), np.float32)
    h_arr = np.zeros((15, M), np.float32)
    c_arr[7:15] = C_loc[:, 1]
    h_arr[7:15] = H_loc[:, 1]
    for j in range(6, -1, -1):
        ch = [2 * j + 1, 2 * j + 2]
        hs2 = h_arr[ch]
        iou = iou_x7[j] + (hs2[0] + hs2[1]) @ W_iouh.T
        i_g, o_g, u_g = np.split(iou, 3)
        i_g, o_g, u_g = sigmoid(i_g), sigmoid(o_g), np.tanh(u_g)
        f = sigmoid(hs2 @ W_fh.T + fx7[j])
        fcs = (f * c_arr[ch]).sum(0)
        c_arr[j] = i_g * u_g + fcs
        h_arr[j] = o_g * np.tanh(c_arr[j])
    return np.stack([c_arr[0], h_arr[0]]).astype(np.float32)


_LAST_IN_MAPS = None


# revision 18
# speedup vs baseline: 1.0612x; 1.0612x over previous
"""ChildSumTreeLSTM on a complete binary tree (N=8191), 8-core Trainium2.

v3: heap-ordered tree = 7 top nodes + 8 independent 1023-node subtrees,
one per NeuronCore. Per core, feature-major [256 feats x cols] layout with
col = subtree-local heap index (level l at cols [2^l, 2^(l+1)), leaves at
[512,1024)).

- x-projections in fp8e4m3 DoubleRow matmuls (K=256 in one instruction),
  computed just-in-time into the same PSUM accumulation group as each
  level's bf16 h-matmuls (scan weights pre-scaled x4096 to match the fp8
  input scaling; activations descale by 1/4096 and add biases via ports).
- f-gate x-terms use a host-duplicated x tensor (x8d[c] = x8[c//2]).
- Input DMA split so the leaf half arrives first; leaf matmuls start
  immediately.
- Dummy matmuls keep the PE busy through activation windows so it stays
  at the fast p-state.
- Device computes leaves + levels 256/128; the top of each subtree
  (<=64, 1023 nodes total + global top 7) finishes on host, vectorized.
"""

import numpy as np

import concourse.bass as bass
import concourse.tile as tile
from concourse import mybir
from concourse.bass_utils import run_bass_kernel_spmd

F32 = mybir.dt.float32
BF16 = mybir.dt.bfloat16
FP8 = mybir.dt.float8e4
AFT = mybir.ActivationFunctionType
DR = mybir.MatmulPerfMode.DoubleRow

N_NODES = 8191
D = 256
M = 256
NCOL = 1024
SUB_LEVELS = 10
DESCALE = 1.0 / 4096.0  # x8 = 128*x, wc8 = 32*Wc, wsc = 4096*W
DEV_LEVELS = (256, 128)  # internal levels computed on device
BOUND = 128              # boundary level emitted to host


def _split_excess_waits(nc, max_waits=1):
    """walrus in this container allows only 1 sync-wait per instruction.

    Tile can attach several; hoist the extras onto injected same-engine NOPs
    immediately preceding the instruction (same blocking semantics)."""
    k = 0
    for f in nc.m.functions:
        for bb in f.blocks:
            out = []
            changed = False
            for ins in bb.instructions:
                si = ins.sync_info
                w = list(si.on_wait) if si and si.on_wait else []
                if len(w) > max_waits:
                    hoist, keep = w[:-max_waits], w[-max_waits:]
                    for sw in hoist:
                        nop = mybir.InstNoOp(name=f"whoist{k}", ins=[], outs=[])
                        k += 1
                        nop.engine = ins.engine
                        nop.sync_info = mybir.SyncInfo(on_wait=[sw], on_update=[])
                        out.append(nop)
                    si.on_wait = keep
                    changed = True
                out.append(ins)
            if changed:
                bb.instructions = out


def _build_module():
    nc = bass.Bass(num_devices=8)

    # head8: wc8 [0:1024] | x8 leaf half [1024:1536] (leaf cols 512..1023)
    head8 = nc.dram_tensor("head8", [128, 2, 1536], FP8, kind="ExternalInput")
    # rest8: x8 internal cols 0..511 [0:512] | x8d cols 128..1023 [512:1408]
    rest8 = nc.dram_tensor("rest8", [128, 2, 1408], FP8, kind="ExternalInput")
    wsc = nc.dram_tensor("wsc", [128, 2, NCOL], BF16, kind="ExternalInput")
    biasd = nc.dram_tensor("biasd", [128, 16], F32, kind="ExternalInput")
    out_c = nc.dram_tensor("out_c", [128, 2 * BOUND], BF16, kind="ExternalOutput")
    out_h = nc.dram_tensor("out_h", [128, 2 * BOUND], BF16, kind="ExternalOutput")

    with tile.TileContext(nc) as tc:
        with (
            tc.tile_pool(name="consts", bufs=1) as consts,
            tc.tile_pool(name="tmps", bufs=2) as tmps,
            tc.tile_pool(name="spsum", bufs=1, space="PSUM") as spsum,
        ):
            sb_h8 = consts.tile([128, 2, 1536], FP8, tag="h8")
            nc.sync.dma_start(out=sb_h8[:], in_=head8[:])
            sb_b = consts.tile([128, 16], F32, tag="bias")
            nc.sync.dma_start(out=sb_b[:], in_=biasd[:])
            sb_r8 = consts.tile([128, 2, 1408], FP8, tag="r8")
            nc.sync.dma_start(out=sb_r8[:], in_=rest8[:])
            sb_wsc = consts.tile([128, 2, NCOL], BF16, tag="wsc")
            nc.sync.dma_start(out=sb_wsc[:], in_=wsc[:])

            wc8 = sb_h8[:, :, 0:1024]
            x8leaf = sb_h8[:, :, 1024:1536]    # leaf cols 512..1023
            x8int = sb_r8[:, :, 0:512]         # cols 0..511

            def x8d_ap(lo, hi):  # duplicated-parent cols lo..hi (128<=lo)
                return sb_r8[:, :, 512 + lo - 128 : 512 + hi - 128]

            H = consts.tile([128, 2, NCOL], BF16, tag="H")
            C = consts.tile([128, 2, NCOL], BF16, tag="C")

            # preload the sigmoid/tanh ACT table during the input DMA
            # (values are garbage; only the table-load side effect matters)
            warm = consts.tile([128, 1], F32, tag="warm")
            nc.scalar.activation(warm[:], H[:, 0, 0:1], AFT.Sigmoid)
            nc.scalar.activation(warm[:], H[:, 0, 0:1], AFT.Tanh)

            # leaf gate tiles (cols 512..1023)
            l_si = consts.tile([128, 2, 512], BF16, tag="l_si")
            l_so = consts.tile([128, 2, 512], BF16, tag="l_so")
            l_tu = consts.tile([128, 2, 512], BF16, tag="l_tu")
            l_fc = consts.tile([128, 2, 512], BF16, tag="l_fc")
            l_iu = consts.tile([128, 2, 512], BF16, tag="l_iu")
            l_tc = consts.tile([128, 2, 512], BF16, tag="l_tc")
            hs = consts.tile([128, 2, 256], BF16, tag="hs_l")

            def ps_iou_tile(tag, name):
                return spsum.tile([128, 2, 256], F32, tag=tag, bufs=2, name=name)

            def ps_f_tile(name):
                return spsum.tile([128, 512], F32, tag="pf", bufs=2, name=name)

            def dummy_mms(count, target_ap):
                # PE p-state keepalive: throwaway fp8 MMs into a PSUM region
                # that the next real group resets with start=True.
                for _ in range(count):
                    nc.tensor.matmul(
                        target_ap, wc8[:, :, 0:128], wc8[:, :, 0:256],
                        start=True, stop=True, perf_mode=DR,
                        skip_group_check=True,
                    )

            # ---- leaf phase ----
            # F: 0,1=i  2,3=o  4,5=u  6,7=fx   (sub = F%2 feature half)
            leaf_ps = {}
            order = (0, 4, 6, 2, 1, 5, 7, 3)  # i0,u0,f0,o0, i1,u1,f1,o1
            for F in order:
                if F < 6:
                    ps = ps_iou_tile(["pi", "pi", "po", "po", "pu", "pu"][F], f"lps{F}")
                    ps_ap = ps[:, :, :]
                else:
                    ps = ps_f_tile(f"lps{F}")
                    ps_ap = ps[:, :]
                nc.tensor.matmul(
                    ps_ap, wc8[:, :, 128 * F : 128 * (F + 1)], x8leaf[:],
                    start=True, stop=True, perf_mode=DR,
                )
                leaf_ps[F] = ps_ap
            gate_of = {0: l_si, 1: l_si, 2: l_so, 3: l_so, 4: l_tu, 5: l_tu,
                       6: l_fc, 7: l_fc}
            func_of = {0: AFT.Sigmoid, 1: AFT.Sigmoid, 2: AFT.Sigmoid,
                       3: AFT.Sigmoid, 4: AFT.Tanh, 5: AFT.Tanh,
                       6: AFT.Sigmoid, 7: AFT.Sigmoid}
            bcol_of = {0: 6, 1: 7, 2: 8, 3: 9, 4: 10, 5: 11, 6: 14, 7: 15}

            def leaf_act(F):
                nc.scalar.activation(
                    gate_of[F][:, F % 2, :], leaf_ps[F], func_of[F],
                    bias=sb_b[:, bcol_of[F] : bcol_of[F] + 1], scale=DESCALE,
                )

            # j=0 chain
            for F in (0, 4, 6):
                leaf_act(F)
            nc.vector.tensor_mul(l_iu[:, 0], l_si[:, 0], l_tu[:, 0])
            nc.vector.tensor_add(C[:, 0, 512:1024], l_iu[:, 0], l_fc[:, 0])
            leaf_act(2)
            nc.scalar.activation(l_tc[:, 0, :], C[:, 0, 512:1024], AFT.Tanh)
            nc.vector.tensor_mul(H[:, 0, 512:1024], l_so[:, 0], l_tc[:, 0])
            nc.vector.tensor_add(
                hs[:, 0, :], H[:, 0, 512:1024:2], H[:, 0, 513:1024:2]
            )
            # j=1 chain (adds on gpsimd to keep DVE free)
            for F in (1, 5, 7):
                leaf_act(F)
            nc.vector.tensor_mul(l_iu[:, 1], l_si[:, 1], l_tu[:, 1])
            nc.vector.tensor_add(C[:, 1, 512:1024], l_iu[:, 1], l_fc[:, 1])
            leaf_act(3)
            nc.scalar.activation(l_tc[:, 1, :], C[:, 1, 512:1024], AFT.Tanh)
            nc.vector.tensor_mul(H[:, 1, 512:1024], l_so[:, 1], l_tc[:, 1])
            nc.vector.tensor_add(
                hs[:, 1, :], H[:, 1, 512:1024:2], H[:, 1, 513:1024:2]
            )

            # ---- internal levels ----
            first = True
            for n in DEV_LEVELS:
                a, b2 = n, 2 * n          # parent cols
                ca, cb = 2 * n, 4 * n     # child cols

                if not first:
                    nc.vector.tensor_add(
                        hs[:, 0, :n], H[:, 0, ca:cb:2], H[:, 0, ca + 1 : cb : 2]
                    )
                    nc.vector.tensor_add(
                        hs[:, 1, :n], H[:, 1, ca:cb:2], H[:, 1, ca + 1 : cb : 2]
                    )

                # JIT x-projections (no H dependency: run during prior acts)
                ps_f = []
                for h in range(2):
                    ps = ps_f_tile(f"psf{h}_{n}")
                    Fb = 768 + 128 * h
                    if h == 0:
                        dummy_mms(12 if first else 10, ps[:, 0:256])
                    nc.tensor.matmul(
                        ps[:, : 2 * n], wc8[:, :, Fb : Fb + 128], x8d_ap(ca, cb),
                        start=True, stop=False, perf_mode=DR,
                    )
                    ps_f.append(ps)
                ps_iou = []
                for pair in range(3):
                    ps = ps_iou_tile(["pi", "po", "pu"][pair], f"ps{'iou'[pair]}_{n}")
                    for sub in range(2):
                        F = 2 * pair + sub
                        nc.tensor.matmul(
                            ps[:, sub, :n],
                            wc8[:, :, 128 * F : 128 * (F + 1)],
                            x8int[:, :, a:b2],
                            start=True, stop=False, perf_mode=DR,
                        )
                    ps_iou.append(ps)

                # h-matmuls: all j=0 first, then j=1 (j=0 leaf chain is ready
                # earlier); within j: f, u, i, o
                for j in range(2):
                    last = j == 1
                    for h in range(2):
                        Fb = 768 + 128 * h
                        nc.tensor.matmul(
                            ps_f[h][:, : 2 * n],
                            sb_wsc[:, j, Fb : Fb + 128],
                            H[:, j, ca:cb],
                            start=False, stop=last,
                        )
                    for pair in (2, 0, 1):  # u, i, o
                        for sub in range(2):
                            F = 2 * pair + sub
                            nc.tensor.matmul(
                                ps_iou[pair][:, sub, :n],
                                sb_wsc[:, j, 128 * F : 128 * (F + 1)],
                                hs[:, j, :n],
                                start=False, stop=last,
                            )

                # acts + cell, sub-split so the c0/tanh_c0/h0 chain runs
                # under the sub-1 and o activations
                t_f = tmps.tile([128, 2, 512], BF16, tag="t_f")
                t_si = tmps.tile([128, 2, 256], BF16, tag="t_si")
                t_so = tmps.tile([128, 2, 256], BF16, tag="t_so")
                t_tu = tmps.tile([128, 2, 256], BF16, tag="t_tu")
                g = tmps.tile([128, 2, 512], BF16, tag="g")
                fc = tmps.tile([128, 2, 256], BF16, tag="fc")
                iu = tmps.tile([128, 2, 256], BF16, tag="iu")
                t_tc = tmps.tile([128, 2, 256], BF16, tag="t_tc")

                def act_f(h):
                    nc.scalar.activation(
                        t_f[:, h, : 2 * n], ps_f[h][:, : 2 * n], AFT.Sigmoid,
                        bias=sb_b[:, 12 + h : 13 + h], scale=DESCALE,
                    )

                def act_iou(pair, gate, func, sub):
                    F = 2 * pair + sub
                    nc.scalar.activation(
                        gate[:, sub, :n], ps_iou[pair][:, sub, :n], func,
                        bias=sb_b[:, F : F + 1], scale=DESCALE,
                    )

                def gfc(s):
                    nc.vector.tensor_mul(
                        g[:, s, : 2 * n], t_f[:, s, : 2 * n], C[:, s, ca:cb]
                    )
                    nc.vector.tensor_add(
                        fc[:, s, :n], g[:, s, 0 : 2 * n : 2], g[:, s, 1 : 2 * n : 2]
                    )

                def iuc(s):
                    nc.vector.tensor_mul(iu[:, s, :n], t_si[:, s, :n], t_tu[:, s, :n])
                    nc.vector.tensor_add(C[:, s, a:b2], iu[:, s, :n], fc[:, s, :n])

                act_f(0)
                act_f(1)
                gfc(0)
                act_iou(2, t_tu, AFT.Tanh, 0)
                act_iou(0, t_si, AFT.Sigmoid, 0)
                gfc(1)
                iuc(0)
                act_iou(2, t_tu, AFT.Tanh, 1)
                act_iou(0, t_si, AFT.Sigmoid, 1)
                iuc(1)
                act_iou(1, t_so, AFT.Sigmoid, 0)
                nc.scalar.activation(t_tc[:, 0, :n], C[:, 0, a:b2], AFT.Tanh)
                nc.vector.tensor_mul(H[:, 0, a:b2], t_so[:, 0, :n], t_tc[:, 0, :n])
                if n == BOUND:
                    nc.sync.dma_start(out=out_c[:, :], in_=C[:, :, BOUND : 2 * BOUND])
                act_iou(1, t_so, AFT.Sigmoid, 1)
                nc.scalar.activation(t_tc[:, 1, :n], C[:, 1, a:b2], AFT.Tanh)
                nc.vector.tensor_mul(H[:, 1, a:b2], t_so[:, 1, :n], t_tc[:, 1, :n])
                if n == BOUND:
                    nc.sync.dma_start(
                        out=out_h[:, 0:BOUND], in_=H[:, 0, BOUND : 2 * BOUND]
                    )
                first = False

            # ---- emit boundary ----
            nc.gpsimd.dma_start(
                out=out_h[:, BOUND : 2 * BOUND], in_=H[:, 1, BOUND : 2 * BOUND]
            )

    _split_excess_waits(nc)
    return nc


_NC_CACHE = None


def _get_module():
    global _NC_CACHE
    if _NC_CACHE is None:
        _NC_CACHE = _build_module()
    return _NC_CACHE


def _expected_children():
    j = (N_NODES - 1) - np.arange(N_NODES)
    internal = (2 * j + 1) < N_NODES
    ch0 = (N_NODES - 1) - (2 * j + 1)
    ch1 = (N_NODES - 1) - (2 * j + 2)
    children = np.stack(
        [np.where(internal, ch0, 0), np.where(internal, ch1, 0)], axis=1
    ).astype(np.int32)
    mask = np.stack([internal, internal], axis=1)
    return children, mask


def _reference_numpy(emb, W_ioux, b_ioux, W_iouh, b_iouh, W_fx, b_fx, W_fh, b_fh,
                     ops, children, child_mask):
    # generic fallback (matches reference.py) for unexpected tree structure
    def sigmoid(v):
        return 1.0 / (1.0 + np.exp(-v))

    N = ops.shape[0]
    Md = W_fh.shape[0]
    x = emb[ops]
    iou_x = x @ W_ioux.T + b_ioux
    fx_all = x @ W_fx.T + b_fx
    ones = np.ones((Md,), np.float32)
    leaf_fh = ones @ W_fh.T + b_fh
    maskf = child_mask.astype(np.float32)
    c_arr = np.zeros((N, Md), np.float32)
    h_arr = np.zeros((N, Md), np.float32)
    for t in range(N):
        idx = children[t]
        m = maskf[t][:, None]
        ch_c = c_arr[idx] * m
        ch_h = h_arr[idx] * m
        is_leaf = maskf[t].sum() == 0
        h_sum = ones if is_leaf else ch_h.sum(0)
        iou = iou_x[t] + h_sum @ W_iouh.T + b_iouh
        i, o, u = np.split(iou, 3)
        i, o, u = sigmoid(i), sigmoid(o), np.tanh(u)
        f = sigmoid(ch_h @ W_fh.T + b_fh + fx_all[t])
        fc_int = (f * ch_c).sum(0)
        fc_leaf = sigmoid(leaf_fh + fx_all[t])
        fc = fc_leaf if is_leaf else fc_int
        c = i * u + fc
        h = o * np.tanh(c)
        c_arr[t] = c
        h_arr[t] = h
    return np.stack([c_arr[N - 1], h_arr[N - 1]])


def _col_index_for_core(k):
    # col 0 pad; cols 1..1023: subtree-local heap order shifted by +1
    # (level l at cols [2^l, 2^(l+1)), leaves exactly at [512, 1024))
    idx = np.zeros(NCOL, np.int64)
    for l in range(SUB_LEVELS):
        n = 1 << l
        g0 = (1 << (3 + l)) - 1 + k * n
        idx[n : 2 * n] = g0 + np.arange(n)
    return idx


def _pack_fm(mat, dtype):
    # mat [cols, 256] -> [128, 2, cols]: out[p, j, c] = mat[c, j*128+p]
    cols = mat.shape[0]
    return np.ascontiguousarray(
        mat.T.reshape(2, 128, cols).transpose(1, 0, 2)
    ).astype(dtype)


def kernel(**inputs):
    emb = np.asarray(inputs["emb"], np.float32)
    W_ioux = np.asarray(inputs["W_ioux"], np.float32)
    b_ioux = np.asarray(inputs["b_ioux"], np.float32)
    W_iouh = np.asarray(inputs["W_iouh"], np.float32)
    b_iouh = np.asarray(inputs["b_iouh"], np.float32)
    W_fx = np.asarray(inputs["W_fx"], np.float32)
    b_fx = np.asarray(inputs["b_fx"], np.float32)
    W_fh = np.asarray(inputs["W_fh"], np.float32)
    b_fh = np.asarray(inputs["b_fh"], np.float32)
    ops = np.asarray(inputs["ops"], np.int32)
    children = np.asarray(inputs["children"], np.int32)
    child_mask = np.asarray(inputs["child_mask"])

    exp_children, exp_mask = _expected_children()
    if (
        ops.shape[0] != N_NODES
        or not np.array_equal(children, exp_children)
        or not np.array_equal(child_mask.astype(bool), exp_mask)
    ):
        return _reference_numpy(
            emb, W_ioux, b_ioux, W_iouh, b_iouh, W_fx, b_fx, W_fh, b_fh,
            ops, children, child_mask,
        )

    import ml_dtypes

    fp8 = ml_dtypes.float8_e4m3
    bf16 = ml_dtypes.bfloat16

    # ---- host prep ----
    x = emb[ops]          # [8191, 256] topo order
    x_heap = x[::-1]      # heap order (topo t = N-1-j)

    Wc = np.concatenate([W_ioux, W_fx], 0)       # [1024, 256]
    Ws = np.concatenate([W_iouh, W_fh], 0)       # [1024, 256]
    wc8 = _pack_fm(32.0 * Wc, fp8)               # [128, 2, 1024]
    wsc = _pack_fm(4096.0 * Ws, bf16)

    bias = np.zeros((128, 16), np.float32)
    bias[:, 0:6] = (b_ioux + b_iouh).reshape(6, 128).T
    bias[:, 6:12] = (b_ioux + W_iouh.sum(1) + b_iouh).reshape(6, 128).T
    bias[:, 12:14] = (b_fx + b_fh).reshape(2, 128).T
    bias[:, 14:16] = (b_fx + W_fh.sum(1) + b_fh).reshape(2, 128).T

    common = {"wsc": wsc, "biasd": bias}
    in_maps = []
    for k in range(8):
        idx = _col_index_for_core(k)
        xv = x_heap[idx]                          # [1024, 256]
        x8 = _pack_fm(128.0 * xv, fp8)
        head8 = np.ascontiguousarray(
            np.concatenate([wc8, x8[:, :, 512:1024]], axis=2)
        )
        x8d = x8[:, :, np.arange(128, 1024) // 2]
        rest8 = np.ascontiguousarray(
            np.concatenate([x8[:, :, 0:512], x8d], axis=2)
        )
        in_maps.append({"head8": head8, "rest8": rest8, **common})

    global _LAST_IN_MAPS
    _LAST_IN_MAPS = in_maps
    nc = _get_module()
    res = run_bass_kernel_spmd(nc, in_maps, list(range(8)))

    # ---- host: subtree levels 64..1 + global top 7 ----
    def sigmoid(v):
        return 1.0 / (1.0 + np.exp(-v))

    # unpack boundary: [128, 2*BOUND] -> [BOUND nodes, 256 feats]
    C_loc = np.zeros((8, 2 * BOUND, M), np.float32)
    H_loc = np.zeros((8, 2 * BOUND, M), np.float32)
    for k in range(8):
        rc = res.results[k]["out_c"]
        rh = res.results[k]["out_h"].astype(np.float32)
        C_loc[k, BOUND:] = rc.reshape(128, 2, BOUND).transpose(2, 1, 0).reshape(BOUND, M)
        H_loc[k, BOUND:] = rh.reshape(128, 2, BOUND).transpose(2, 1, 0).reshape(BOUND, M)

    # x-projections for host nodes (cols 1..BOUND-1 per core + global top 7)
    nb = BOUND - 1
    idx_all = np.stack([_col_index_for_core(k)[1:BOUND] for k in range(8)])
    x_host = x_heap[idx_all.reshape(-1)].astype(np.float32)
    iou_xh = (x_host @ W_ioux.T + b_ioux + b_iouh).reshape(8, nb, 3 * M)
    fx_h = (x_host @ W_fx.T + b_fx + b_fh).reshape(8, nb, M)

    n = BOUND // 2
    while n >= 1:
        ch_h = H_loc[:, 2 * n : 4 * n]            # [8, 2n, 256]
        ch_c = C_loc[:, 2 * n : 4 * n]
        hsum = ch_h[:, 0::2] + ch_h[:, 1::2]      # [8, n, 256]
        iou = iou_xh[:, n - 1 : 2 * n - 1] + hsum @ W_iouh.T
        i_g = sigmoid(iou[:, :, :M])
        o_g = sigmoid(iou[:, :, M : 2 * M])
        u_g = np.tanh(iou[:, :, 2 * M :])
        fxd = np.repeat(fx_h[:, n - 1 : 2 * n - 1], 2, axis=1)
        f = sigmoid(ch_h @ W_fh.T + fxd)
        gfc = f * ch_c
        fcs = gfc[:, 0::2] + gfc[:, 1::2]
        c = i_g * u_g + fcs
        C_loc[:, n : 2 * n] = c
        H_loc[:, n : 2 * n] = o_g * np.tanh(c)
        n //= 2

    # global top 15: nodes 7..14 are the subtree roots (core k -> 7+k)
    x_top = x_heap[0:7].astype(np.float32)
    iou_x7 = x_top @ W_ioux.T + b_ioux + b_iouh
    fx7 = x_top @ W_fx.T + b_fx + b_fh
    c_arr = np.zeros((15, M), np.float32)
    h_arr = np.zeros((15, M), np.float32)
    c_arr[7:15] = C_loc[:, 1]
    h_arr[7:15] = H_loc[:, 1]
    for j in range(6, -1, -1):
        ch = [2 * j + 1, 2 * j + 2]
        hs2 = h_arr[ch]
        iou = iou_x7[j] + (hs2[0] + hs2[1]) @ W_iouh.T
        i_g, o_g, u_g = np.split(iou, 3)
        i_g, o_g, u_g = sigmoid(i_g), sigmoid(o_g), np.tanh(u_g)
        f = sigmoid(hs2 @ W_fh.T + fx7[j])
        fcs = (f * c_arr[ch]).sum(0)
        c_arr[j] = i_g * u_g + fcs
        h_arr[j] = o_g * np.tanh(c_arr[j])
    return np.stack([c_arr[0], h_arr[0]]).astype(np.float32)


_LAST_IN_MAPS = None


# revision 19
# speedup vs baseline: 1.2033x; 1.1339x over previous
"""ChildSumTreeLSTM on a complete binary tree (N=8191), 8-core Trainium2.

v3: heap-ordered tree = 7 top nodes + 8 independent 1023-node subtrees,
one per NeuronCore. Per core, feature-major [256 feats x cols] layout with
col = subtree-local heap index (level l at cols [2^l, 2^(l+1)), leaves at
[512,1024)).

- x-projections in fp8e4m3 DoubleRow matmuls (K=256 in one instruction),
  computed just-in-time into the same PSUM accumulation group as each
  level's bf16 h-matmuls (scan weights pre-scaled x4096 to match the fp8
  input scaling; activations descale by 1/4096 and add biases via ports).
- f-gate x-terms use a host-duplicated x tensor (x8d[c] = x8[c//2]).
- Input DMA split so the leaf half arrives first; leaf matmuls start
  immediately.
- Dummy matmuls keep the PE busy through activation windows so it stays
  at the fast p-state.
- Device computes leaves + levels 256/128; the top of each subtree
  (<=64, 1023 nodes total + global top 7) finishes on host, vectorized.
"""

import numpy as np

import concourse.bass as bass
import concourse.tile as tile
from concourse import mybir
from concourse.bass_utils import run_bass_kernel_spmd

F32 = mybir.dt.float32
BF16 = mybir.dt.bfloat16
FP8 = mybir.dt.float8e4
AFT = mybir.ActivationFunctionType
DR = mybir.MatmulPerfMode.DoubleRow

N_NODES = 8191
D = 256
M = 256
NCOL = 1024
SUB_LEVELS = 10
DESCALE = 1.0 / 4096.0  # x8 = 128*x, wc8 = 32*Wc, wsc = 4096*W
DEV_LEVELS = (256,)      # internal levels computed on device
BOUND = 256              # boundary level emitted to host


def _split_excess_waits(nc, max_waits=1):
    """walrus in this container allows only 1 sync-wait per instruction.

    Tile can attach several; hoist the extras onto injected same-engine NOPs
    immediately preceding the instruction (same blocking semantics)."""
    k = 0
    for f in nc.m.functions:
        for bb in f.blocks:
            out = []
            changed = False
            for ins in bb.instructions:
                si = ins.sync_info
                w = list(si.on_wait) if si and si.on_wait else []
                if len(w) > max_waits:
                    hoist, keep = w[:-max_waits], w[-max_waits:]
                    for sw in hoist:
                        nop = mybir.InstNoOp(name=f"whoist{k}", ins=[], outs=[])
                        k += 1
                        nop.engine = ins.engine
                        nop.sync_info = mybir.SyncInfo(on_wait=[sw], on_update=[])
                        out.append(nop)
                    si.on_wait = keep
                    changed = True
                out.append(ins)
            if changed:
                bb.instructions = out


def _build_module():
    nc = bass.Bass(num_devices=8)

    # head8: wc8 [0:1024] | x8 leaf half [1024:1536] (leaf cols 512..1023)
    head8 = nc.dram_tensor("head8", [128, 2, 1536], FP8, kind="ExternalInput")
    # rest8: x8 internal cols 0..511 [0:512] | x8d cols 128..1023 [512:1408]
    rest8 = nc.dram_tensor("rest8", [128, 2, 1408], FP8, kind="ExternalInput")
    wsc = nc.dram_tensor("wsc", [128, 2, NCOL], BF16, kind="ExternalInput")
    biasd = nc.dram_tensor("biasd", [128, 16], F32, kind="ExternalInput")
    out_c = nc.dram_tensor("out_c", [128, 2 * BOUND], BF16, kind="ExternalOutput")
    out_h = nc.dram_tensor("out_h", [128, 2 * BOUND], BF16, kind="ExternalOutput")

    with tile.TileContext(nc) as tc:
        with (
            tc.tile_pool(name="consts", bufs=1) as consts,
            tc.tile_pool(name="tmps", bufs=2) as tmps,
            tc.tile_pool(name="spsum", bufs=1, space="PSUM") as spsum,
        ):
            sb_h8 = consts.tile([128, 2, 1536], FP8, tag="h8")
            nc.sync.dma_start(out=sb_h8[:], in_=head8[:])
            sb_b = consts.tile([128, 16], F32, tag="bias")
            nc.sync.dma_start(out=sb_b[:], in_=biasd[:])
            sb_r8 = consts.tile([128, 2, 1408], FP8, tag="r8")
            nc.sync.dma_start(out=sb_r8[:], in_=rest8[:])
            sb_wsc = consts.tile([128, 2, NCOL], BF16, tag="wsc")
            nc.sync.dma_start(out=sb_wsc[:], in_=wsc[:])

            wc8 = sb_h8[:, :, 0:1024]
            x8leaf = sb_h8[:, :, 1024:1536]    # leaf cols 512..1023
            x8int = sb_r8[:, :, 0:512]         # cols 0..511

            def x8d_ap(lo, hi):  # duplicated-parent cols lo..hi (128<=lo)
                return sb_r8[:, :, 512 + lo - 128 : 512 + hi - 128]

            H = consts.tile([128, 2, NCOL], BF16, tag="H")
            C = consts.tile([128, 2, NCOL], BF16, tag="C")

            # preload the sigmoid/tanh ACT table during the input DMA
            # (values are garbage; only the table-load side effect matters)
            warm = consts.tile([128, 1], F32, tag="warm")
            nc.scalar.activation(warm[:], H[:, 0, 0:1], AFT.Sigmoid)
            nc.scalar.activation(warm[:], H[:, 0, 0:1], AFT.Tanh)

            # leaf gate tiles (cols 512..1023)
            l_si = consts.tile([128, 2, 512], BF16, tag="l_si")
            l_so = consts.tile([128, 2, 512], BF16, tag="l_so")
            l_tu = consts.tile([128, 2, 512], BF16, tag="l_tu")
            l_fc = consts.tile([128, 2, 512], BF16, tag="l_fc")
            l_iu = consts.tile([128, 2, 512], BF16, tag="l_iu")
            l_tc = consts.tile([128, 2, 512], BF16, tag="l_tc")
            hs = consts.tile([128, 2, 256], BF16, tag="hs_l")

            def ps_iou_tile(tag, name):
                return spsum.tile([128, 2, 256], F32, tag=tag, bufs=2, name=name)

            def ps_f_tile(name):
                return spsum.tile([128, 512], F32, tag="pf", bufs=2, name=name)

            def dummy_mms(count, target_ap):
                # PE p-state keepalive: throwaway fp8 MMs into a PSUM region
                # that the next real group resets with start=True.
                for _ in range(count):
                    nc.tensor.matmul(
                        target_ap, wc8[:, :, 0:128], wc8[:, :, 0:256],
                        start=True, stop=True, perf_mode=DR,
                        skip_group_check=True,
                    )

            # ---- leaf phase ----
            # F: 0,1=i  2,3=o  4,5=u  6,7=fx   (sub = F%2 feature half)
            leaf_ps = {}
            order = (0, 4, 6, 2, 1, 5, 7, 3)  # i0,u0,f0,o0, i1,u1,f1,o1
            for F in order:
                if F < 6:
                    ps = ps_iou_tile(["pi", "pi", "po", "po", "pu", "pu"][F], f"lps{F}")
                    ps_ap = ps[:, :, :]
                else:
                    ps = ps_f_tile(f"lps{F}")
                    ps_ap = ps[:, :]
                nc.tensor.matmul(
                    ps_ap, wc8[:, :, 128 * F : 128 * (F + 1)], x8leaf[:],
                    start=True, stop=True, perf_mode=DR,
                )
                leaf_ps[F] = ps_ap
            gate_of = {0: l_si, 1: l_si, 2: l_so, 3: l_so, 4: l_tu, 5: l_tu,
                       6: l_fc, 7: l_fc}
            func_of = {0: AFT.Sigmoid, 1: AFT.Sigmoid, 2: AFT.Sigmoid,
                       3: AFT.Sigmoid, 4: AFT.Tanh, 5: AFT.Tanh,
                       6: AFT.Sigmoid, 7: AFT.Sigmoid}
            bcol_of = {0: 6, 1: 7, 2: 8, 3: 9, 4: 10, 5: 11, 6: 14, 7: 15}

            def leaf_act(F):
                nc.scalar.activation(
                    gate_of[F][:, F % 2, :], leaf_ps[F], func_of[F],
                    bias=sb_b[:, bcol_of[F] : bcol_of[F] + 1], scale=DESCALE,
                )

            # j=0 chain
            for F in (0, 4, 6):
                leaf_act(F)
            nc.vector.tensor_mul(l_iu[:, 0], l_si[:, 0], l_tu[:, 0])
            nc.vector.tensor_add(C[:, 0, 512:1024], l_iu[:, 0], l_fc[:, 0])
            leaf_act(2)
            nc.scalar.activation(l_tc[:, 0, :], C[:, 0, 512:1024], AFT.Tanh)
            nc.vector.tensor_mul(H[:, 0, 512:1024], l_so[:, 0], l_tc[:, 0])
            nc.vector.tensor_add(
                hs[:, 0, :], H[:, 0, 512:1024:2], H[:, 0, 513:1024:2]
            )
            # j=1 chain (adds on gpsimd to keep DVE free)
            for F in (1, 5, 7):
                leaf_act(F)
            nc.vector.tensor_mul(l_iu[:, 1], l_si[:, 1], l_tu[:, 1])
            nc.vector.tensor_add(C[:, 1, 512:1024], l_iu[:, 1], l_fc[:, 1])
            leaf_act(3)
            nc.scalar.activation(l_tc[:, 1, :], C[:, 1, 512:1024], AFT.Tanh)
            nc.vector.tensor_mul(H[:, 1, 512:1024], l_so[:, 1], l_tc[:, 1])
            nc.vector.tensor_add(
                hs[:, 1, :], H[:, 1, 512:1024:2], H[:, 1, 513:1024:2]
            )

            # ---- internal levels ----
            first = True
            for n in DEV_LEVELS:
                a, b2 = n, 2 * n          # parent cols
                ca, cb = 2 * n, 4 * n     # child cols

                if not first:
                    nc.vector.tensor_add(
                        hs[:, 0, :n], H[:, 0, ca:cb:2], H[:, 0, ca + 1 : cb : 2]
                    )
                    nc.vector.tensor_add(
                        hs[:, 1, :n], H[:, 1, ca:cb:2], H[:, 1, ca + 1 : cb : 2]
                    )

                # JIT x-projections (no H dependency: run during prior acts)
                ps_f = []
                for h in range(2):
                    ps = ps_f_tile(f"psf{h}_{n}")
                    Fb = 768 + 128 * h
                    if h == 0:
                        dummy_mms(12 if first else 10, ps[:, 0:256])
                    nc.tensor.matmul(
                        ps[:, : 2 * n], wc8[:, :, Fb : Fb + 128], x8d_ap(ca, cb),
                        start=True, stop=False, perf_mode=DR,
                    )
                    ps_f.append(ps)
                ps_iou = []
                for pair in range(3):
                    ps = ps_iou_tile(["pi", "po", "pu"][pair], f"ps{'iou'[pair]}_{n}")
                    for sub in range(2):
                        F = 2 * pair + sub
                        nc.tensor.matmul(
                            ps[:, sub, :n],
                            wc8[:, :, 128 * F : 128 * (F + 1)],
                            x8int[:, :, a:b2],
                            start=True, stop=False, perf_mode=DR,
                        )
                    ps_iou.append(ps)

                # h-matmuls: all j=0 first, then j=1 (j=0 leaf chain is ready
                # earlier); within j: f, u, i, o
                for j in range(2):
                    last = j == 1
                    for h in range(2):
                        Fb = 768 + 128 * h
                        nc.tensor.matmul(
                            ps_f[h][:, : 2 * n],
                            sb_wsc[:, j, Fb : Fb + 128],
                            H[:, j, ca:cb],
                            start=False, stop=last,
                        )
                    for pair in (2, 0, 1):  # u, i, o
                        for sub in range(2):
                            F = 2 * pair + sub
                            nc.tensor.matmul(
                                ps_iou[pair][:, sub, :n],
                                sb_wsc[:, j, 128 * F : 128 * (F + 1)],
                                hs[:, j, :n],
                                start=False, stop=last,
                            )

                # acts + cell, sub-split so the c0/tanh_c0/h0 chain runs
                # under the sub-1 and o activations
                t_f = tmps.tile([128, 2, 512], BF16, tag="t_f")
                t_si = tmps.tile([128, 2, 256], BF16, tag="t_si")
                t_so = tmps.tile([128, 2, 256], BF16, tag="t_so")
                t_tu = tmps.tile([128, 2, 256], BF16, tag="t_tu")
                g = tmps.tile([128, 2, 512], BF16, tag="g")
                fc = tmps.tile([128, 2, 256], BF16, tag="fc")
                iu = tmps.tile([128, 2, 256], BF16, tag="iu")
                t_tc = tmps.tile([128, 2, 256], BF16, tag="t_tc")

                def act_f(h):
                    nc.scalar.activation(
                        t_f[:, h, : 2 * n], ps_f[h][:, : 2 * n], AFT.Sigmoid,
                        bias=sb_b[:, 12 + h : 13 + h], scale=DESCALE,
                    )

                def act_iou(pair, gate, func, sub):
                    F = 2 * pair + sub
                    nc.scalar.activation(
                        gate[:, sub, :n], ps_iou[pair][:, sub, :n], func,
                        bias=sb_b[:, F : F + 1], scale=DESCALE,
                    )

                def gfc(s):
                    nc.vector.tensor_mul(
                        g[:, s, : 2 * n], t_f[:, s, : 2 * n], C[:, s, ca:cb]
                    )
                    nc.vector.tensor_add(
                        fc[:, s, :n], g[:, s, 0 : 2 * n : 2], g[:, s, 1 : 2 * n : 2]
                    )

                def iuc(s):
                    nc.vector.tensor_mul(iu[:, s, :n], t_si[:, s, :n], t_tu[:, s, :n])
                    nc.vector.tensor_add(C[:, s, a:b2], iu[:, s, :n], fc[:, s, :n])

                act_f(0)
                act_f(1)
                gfc(0)
                act_iou(2, t_tu, AFT.Tanh, 0)
                act_iou(0, t_si, AFT.Sigmoid, 0)
                gfc(1)
                iuc(0)
                act_iou(2, t_tu, AFT.Tanh, 1)
                act_iou(0, t_si, AFT.Sigmoid, 1)
                iuc(1)
                act_iou(1, t_so, AFT.Sigmoid, 0)
                nc.scalar.activation(t_tc[:, 0, :n], C[:, 0, a:b2], AFT.Tanh)
                nc.vector.tensor_mul(H[:, 0, a:b2], t_so[:, 0, :n], t_tc[:, 0, :n])
                if n == BOUND:
                    nc.sync.dma_start(out=out_c[:, :], in_=C[:, :, BOUND : 2 * BOUND])
                act_iou(1, t_so, AFT.Sigmoid, 1)
                nc.scalar.activation(t_tc[:, 1, :n], C[:, 1, a:b2], AFT.Tanh)
                nc.vector.tensor_mul(H[:, 1, a:b2], t_so[:, 1, :n], t_tc[:, 1, :n])
                if n == BOUND:
                    nc.sync.dma_start(
                        out=out_h[:, 0:BOUND], in_=H[:, 0, BOUND : 2 * BOUND]
                    )
                first = False

            # ---- emit boundary ----
            nc.gpsimd.dma_start(
                out=out_h[:, BOUND : 2 * BOUND], in_=H[:, 1, BOUND : 2 * BOUND]
            )

    _split_excess_waits(nc)
    return nc


_NC_CACHE = None


def _get_module():
    global _NC_CACHE
    if _NC_CACHE is None:
        _NC_CACHE = _build_module()
    return _NC_CACHE


def _expected_children():
    j = (N_NODES - 1) - np.arange(N_NODES)
    internal = (2 * j + 1) < N_NODES
    ch0 = (N_NODES - 1) - (2 * j + 1)
    ch1 = (N_NODES - 1) - (2 * j + 2)
    children = np.stack(
        [np.where(internal, ch0, 0), np.where(internal, ch1, 0)], axis=1
    ).astype(np.int32)
    mask = np.stack([internal, internal], axis=1)
    return children, mask


def _reference_numpy(emb, W_ioux, b_ioux, W_iouh, b_iouh, W_fx, b_fx, W_fh, b_fh,
                     ops, children, child_mask):
    # generic fallback (matches reference.py) for unexpected tree structure
    def sigmoid(v):
        return 1.0 / (1.0 + np.exp(-v))

    N = ops.shape[0]
    Md = W_fh.shape[0]
    x = emb[ops]
    iou_x = x @ W_ioux.T + b_ioux
    fx_all = x @ W_fx.T + b_fx
    ones = np.ones((Md,), np.float32)
    leaf_fh = ones @ W_fh.T + b_fh
    maskf = child_mask.astype(np.float32)
    c_arr = np.zeros((N, Md), np.float32)
    h_arr = np.zeros((N, Md), np.float32)
    for t in range(N):
        idx = children[t]
        m = maskf[t][:, None]
        ch_c = c_arr[idx] * m
        ch_h = h_arr[idx] * m
        is_leaf = maskf[t].sum() == 0
        h_sum = ones if is_leaf else ch_h.sum(0)
        iou = iou_x[t] + h_sum @ W_iouh.T + b_iouh
        i, o, u = np.split(iou, 3)
        i, o, u = sigmoid(i), sigmoid(o), np.tanh(u)
        f = sigmoid(ch_h @ W_fh.T + b_fh + fx_all[t])
        fc_int = (f * ch_c).sum(0)
        fc_leaf = sigmoid(leaf_fh + fx_all[t])
        fc = fc_leaf if is_leaf else fc_int
        c = i * u + fc
        h = o * np.tanh(c)
        c_arr[t] = c
        h_arr[t] = h
    return np.stack([c_arr[N - 1], h_arr[N - 1]])


def _col_index_for_core(k):
    # col 0 pad; cols 1..1023: subtree-local heap order shifted by +1
    # (level l at cols [2^l, 2^(l+1)), leaves exactly at [512, 1024))
    idx = np.zeros(NCOL, np.int64)
    for l in range(SUB_LEVELS):
        n = 1 << l
        g0 = (1 << (3 + l)) - 1 + k * n
        idx[n : 2 * n] = g0 + np.arange(n)
    return idx


def _pack_fm(mat, dtype):
    # mat [cols, 256] -> [128, 2, cols]: out[p, j, c] = mat[c, j*128+p]
    cols = mat.shape[0]
    return np.ascontiguousarray(
        mat.T.reshape(2, 128, cols).transpose(1, 0, 2)
    ).astype(dtype)


def kernel(**inputs):
    emb = np.asarray(inputs["emb"], np.float32)
    W_ioux = np.asarray(inputs["W_ioux"], np.float32)
    b_ioux = np.asarray(inputs["b_ioux"], np.float32)
    W_iouh = np.asarray(inputs["W_iouh"], np.float32)
    b_iouh = np.asarray(inputs["b_iouh"], np.float32)
    W_fx = np.asarray(inputs["W_fx"], np.float32)
    b_fx = np.asarray(inputs["b_fx"], np.float32)
    W_fh = np.asarray(inputs["W_fh"], np.float32)
    b_fh = np.asarray(inputs["b_fh"], np.float32)
    ops = np.asarray(inputs["ops"], np.int32)
    children = np.asarray(inputs["children"], np.int32)
    child_mask = np.asarray(inputs["child_mask"])

    exp_children, exp_mask = _expected_children()
    if (
        ops.shape[0] != N_NODES
        or not np.array_equal(children, exp_children)
        or not np.array_equal(child_mask.astype(bool), exp_mask)
    ):
        return _reference_numpy(
            emb, W_ioux, b_ioux, W_iouh, b_iouh, W_fx, b_fx, W_fh, b_fh,
            ops, children, child_mask,
        )

    import ml_dtypes

    fp8 = ml_dtypes.float8_e4m3
    bf16 = ml_dtypes.bfloat16

    # ---- host prep ----
    x = emb[ops]          # [8191, 256] topo order
    x_heap = x[::-1]      # heap order (topo t = N-1-j)

    Wc = np.concatenate([W_ioux, W_fx], 0)       # [1024, 256]
    Ws = np.concatenate([W_iouh, W_fh], 0)       # [1024, 256]
    wc8 = _pack_fm(32.0 * Wc, fp8)               # [128, 2, 1024]
    wsc = _pack_fm(4096.0 * Ws, bf16)

    bias = np.zeros((128, 16), np.float32)
    bias[:, 0:6] = (b_ioux + b_iouh).reshape(6, 128).T
    bias[:, 6:12] = (b_ioux + W_iouh.sum(1) + b_iouh).reshape(6, 128).T
    bias[:, 12:14] = (b_fx + b_fh).reshape(2, 128).T
    bias[:, 14:16] = (b_fx + W_fh.sum(1) + b_fh).reshape(2, 128).T

    common = {"wsc": wsc, "biasd": bias}
    in_maps = []
    for k in range(8):
        idx = _col_index_for_core(k)
        xv = x_heap[idx]                          # [1024, 256]
        x8 = _pack_fm(128.0 * xv, fp8)
        head8 = np.ascontiguousarray(
            np.concatenate([wc8, x8[:, :, 512:1024]], axis=2)
        )
        x8d = x8[:, :, np.arange(128, 1024) // 2]
        rest8 = np.ascontiguousarray(
            np.concatenate([x8[:, :, 0:512], x8d], axis=2)
        )
        in_maps.append({"head8": head8, "rest8": rest8, **common})

    global _LAST_IN_MAPS
    _LAST_IN_MAPS = in_maps
    nc = _get_module()
    res = run_bass_kernel_spmd(nc, in_maps, list(range(8)))

    # ---- host: subtree levels 64..1 + global top 7 ----
    def sigmoid(v):
        return 1.0 / (1.0 + np.exp(-v))

    # unpack boundary: [128, 2*BOUND] -> [BOUND nodes, 256 feats]
    C_loc = np.zeros((8, 2 * BOUND, M), np.float32)
    H_loc = np.zeros((8, 2 * BOUND, M), np.float32)
    for k in range(8):
        rc = res.results[k]["out_c"]
        rh = res.results[k]["out_h"].astype(np.float32)
        C_loc[k, BOUND:] = rc.reshape(128, 2, BOUND).transpose(2, 1, 0).reshape(BOUND, M)
        H_loc[k, BOUND:] = rh.reshape(128, 2, BOUND).transpose(2, 1, 0).reshape(BOUND, M)

    # x-projections for host nodes (cols 1..BOUND-1 per core + global top 7)
    nb = BOUND - 1
    idx_all = np.stack([_col_index_for_core(k)[1:BOUND] for k in range(8)])
    x_host = x_heap[idx_all.reshape(-1)].astype(np.float32)
    iou_xh = (x_host @ W_ioux.T + b_ioux + b_iouh).reshape(8, nb, 3 * M)
    fx_h = (x_host @ W_fx.T + b_fx + b_fh).reshape(8, nb, M)

    n = BOUND // 2
    while n >= 1:
        ch_h = H_loc[:, 2 * n : 4 * n]            # [8, 2n, 256]
        ch_c = C_loc[:, 2 * n : 4 * n]
        hsum = ch_h[:, 0::2] + ch_h[:, 1::2]      # [8, n, 256]
        iou = iou_xh[:, n - 1 : 2 * n - 1] + hsum @ W_iouh.T
        i_g = sigmoid(iou[:, :, :M])
        o_g = sigmoid(iou[:, :, M : 2 * M])
        u_g = np.tanh(iou[:, :, 2 * M :])
        fxd = np.repeat(fx_h[:, n - 1 : 2 * n - 1], 2, axis=1)
        f = sigmoid(ch_h @ W_fh.T + fxd)
        gfc = f * ch_c
        fcs = gfc[:, 0::2] + gfc[:, 1::2]
        c = i_g * u_g + fcs
        C_loc[:, n : 2 * n] = c
        H_loc[:, n : 2 * n] = o_g * np.tanh(c)
        n //= 2

    # global top 15: nodes 7..14 are the subtree roots (core k -> 7+k)
    x_top = x_heap[0:7].astype(np.float32)
    iou_x7 = x_top @ W_ioux.T + b_ioux + b_iouh
    fx7 = x_top @ W_fx.T + b_fx + b_fh
    c_arr = np.zeros((15, M), np.float32)
    h_arr = np.zeros((15, M), np.float32)
    c_arr[7:15] = C_loc[:, 1]
    h_arr[7:15] = H_loc[:, 1]
    for j in range(6, -1, -1):
        ch = [2 * j + 1, 2 * j + 2]
        hs2 = h_arr[ch]
        iou = iou_x7[j] + (hs2[0] + hs2[1]) @ W_iouh.T
        i_g, o_g, u_g = np.split(iou, 3)
        i_g, o_g, u_g = sigmoid(i_g), sigmoid(o_g), np.tanh(u_g)
        f = sigmoid(hs2 @ W_fh.T + fx7[j])
        fcs = (f * c_arr[ch]).sum(0)
        c_arr[j] = i_g * u_g + fcs
        h_arr[j] = o_g * np.tanh(c_arr[j])
    return np.stack([c_arr[0], h_arr[0]]).astype(np.float32)


_LAST_IN_MAPS = None
